# revision 23
# baseline (speedup 1.0000x reference)
"""Multi-head self-attention on 8 Trainium2 NeuronCores.

Problem: X[2,2048,2048] -> MHA(16 heads, head_dim 128) -> [2,2048,2048].

Sharding: core c in 0..7 handles batch b = c // 4 and head-group g = c % 4
(4 heads = 512 hidden columns per core).  Each core computes
    Q^T,K^T,V  (its 512-column slice of the QKV projections)
    per-head attention (softmax without max-subtraction; inputs are bounded)
    partial output projection  out_heads @ Wo[512-slice, :]  -> [2048, 2048]
The 4 partial projections per batch are summed on the host (the tensor-
parallel "all-reduce" is done in numpy) and the output bias is added there.

The PE is pure column-rate-bound (~0.49 ns/col sustained; LDWEIGHTS and
per-instruction overhead fully hidden), so the schedule is built to keep
the PE column stream dense from t~0:

  - ~10 warm-up matmuls on memset data run while the first DMA chunks land,
    so the HAM clock-gate reaches 8/8 before real work starts.
  - Phase A starts with a "chase" pass: 8 K^T groups (heads 0,1 x all four
    S-chunks) accumulate in 8 PSUM banks with the contraction (k) loop
    OUTERMOST, so every arriving (wk,xt) chunk immediately feeds 8 matmuls.
    PE never has to wait for the full 10.5 MB input load.
  - Softmax denominators: an incremental DVE pair-add chase over the exp
    tiles followed by ONE ones-matmul per unit, emitted AFTER the AV
    accumulation (the tree never sits on the PE critical path).
  - Phase B runs as a lag-1 software pipeline: unit (c,h)'s 16 score
    matmuls are woven with the previous unit's 16 AV matmuls, so the PE
    never waits on the exp stream through the 3-deep PSUM rotation.
  - Phase C (output projection) tiles for chunk c-1 are emitted one unit
    later as additional PE filler.
"""

import math
import sys
from contextlib import ExitStack

import numpy as np

sys.path.insert(0, "/opt/trn_rl_repo")

import ml_dtypes  # noqa: E402

import concourse.bass as bass  # noqa: E402
import concourse.mybir as mybir  # noqa: E402
import concourse.tile as tile  # noqa: E402
from concourse import bacc  # noqa: E402

B, S, H = 2, 2048, 2048
HEADS, D = 16, 128
NC = 8
GROUPS = 4            # cores per batch (head-group parallel)
CW = H // GROUPS      # 512 hidden columns per core (4 heads)
HG = CW // D          # 4 heads per core
P = 128               # partitions
FN = 512              # matmul free-dim / psum bank (f32)
KT = H // P           # 16 contraction tiles for the projections
SQ = S // FN          # 4 query chunks of 512
SK = S // P           # 16 key tiles of 128
NWARM = 10            # HAM warm-up filler matmuls

BF16 = mybir.dt.bfloat16
F32 = mybir.dt.float32

_CACHE = {}


def _build_nc():
    nc = bacc.Bacc()
    xt = nc.dram_tensor("xt", [H, S], BF16, kind="ExternalInput")
    wq = nc.dram_tensor("wq", [H, CW], BF16, kind="ExternalInput")
    wk = nc.dram_tensor("wk", [H, CW], BF16, kind="ExternalInput")
    wv = nc.dram_tensor("wv", [H, CW], BF16, kind="ExternalInput")
    wo = nc.dram_tensor("wo", [CW, H], BF16, kind="ExternalInput")
    bq = nc.dram_tensor("bq", [CW], F32, kind="ExternalInput")
    bk = nc.dram_tensor("bk", [CW], F32, kind="ExternalInput")
    bv = nc.dram_tensor("bv", [CW], F32, kind="ExternalInput")
    y = nc.dram_tensor("y", [S, H], BF16, kind="ExternalOutput")

    with tile.TileContext(nc) as tc:
        _emit(nc, tc, xt[:], wq[:], wk[:], wv[:], wo[:], bq[:], bk[:], bv[:], y[:])
    nc.finalize()
    return nc


def _emit(nc, tc, xt, wq, wk, wv, wo, bq, bk, bv, y):
    with ExitStack() as ctx:
        consts = ctx.enter_context(tc.tile_pool(name="consts", bufs=1))
        qkv = ctx.enter_context(tc.tile_pool(name="qkv", bufs=1))
        wo_pool = ctx.enter_context(tc.tile_pool(name="wo", bufs=1))
        # shared PSUM pool: tag "ps" rotates 3x [P, 2, FN] tiles (6 banks);
        # tag "av" double-buffers the 1-bank AV accumulator (2 banks)
        psum = ctx.enter_context(tc.tile_pool(name="psum", bufs=3, space="PSUM"))

        # --- constants ------------------------------------------------------
        ones_s = consts.tile([P, P], BF16)
        nc.vector.memset(ones_s, 1.0)
        warm_rhs = consts.tile([P, FN], BF16)
        nc.vector.memset(warm_rhs, 0.001)
        bq_s = consts.tile([P, HG], F32)
        bk_s = consts.tile([P, HG], F32)
        with nc.allow_non_contiguous_dma(reason="tiny one-time bias load"):
            nc.gpsimd.dma_start(bq_s, bq.rearrange("(m p) -> p m", p=P))
            nc.gpsimd.dma_start(bk_s, bk.rearrange("(m p) -> p m", p=P))
        bv_row = consts.tile([P, CW], F32)
        nc.gpsimd.dma_start(bv_row, bv[None, :].to_broadcast([P, CW]))

        # --- HAM warm-up fillers (run while the first DMA chunks land) -----
        warm_ps = psum.tile([P, 2, FN], F32, tag="ps", name="warm_ps")
        for i in range(NWARM):
            nc.tensor.matmul(warm_ps[:, i % 2], ones_s, warm_rhs,
                             start=True, stop=True)

        qt = qkv.tile([P, HG, S], BF16)     # Q^T: [d-part, head, S]
        kt_sb = qkv.tile([P, HG, S], BF16)  # K^T
        v_sb = qkv.tile([P, SK, CW], BF16)  # V: [S-part(tile), S-tile, 4*d]
        et0 = qkv.tile([P, SK, FN], BF16)   # exp tile for unit (c=0, h=0)
        wo_s = wo_pool.tile([P, HG, H], BF16)

        # phase-B pools are opened lazily (SBUF is tight during phase A);
        # these names are captured by the closures below
        exp_pool = red = rden_pool = ystage = outt = None

        def finish_av(ps_av, pc, ph, pdsum):
            """ones-matmul + reciprocal + normalize for a completed AV chain."""
            ps_one = psum.tile([P, 2, FN], F32, tag="ps", name="ps_one")
            nc.tensor.matmul(ps_one[:, 0], ones_s, pdsum, start=True, stop=True)
            rden = rden_pool.tile([P, FN], F32, name="rden")
            nc.vector.reciprocal_approx_fast(out=rden, in_=ps_one[:, 0])
            nc.vector.tensor_mul(
                outt[:, ph, pc * FN:(pc + 1) * FN], ps_av, rden)

        def emit_score_pair(c, h, j, et):
            ps = psum.tile([P, 2, FN], F32, tag="ps", name="ps_s")
            for i in range(2):
                nc.tensor.matmul(
                    ps[:, i],
                    kt_sb[:, h, (2 * j + i) * P:(2 * j + i + 1) * P],
                    qt[:, h, c * FN:(c + 1) * FN],
                    start=True, stop=True,
                )
            nc.scalar.activation(
                et[:, 2 * j:2 * j + 2, :].rearrange("p a q -> p (a q)"),
                ps.rearrange("p a q -> p (a q)"),
                mybir.ActivationFunctionType.Exp)

        def emit_tree(et):
            """incremental DVE denominator tree over a full exp tile"""
            u0 = red.tile([P, 4, FN], BF16, tag="u", bufs=2, name="u0")
            nc.vector.tensor_add(u0, et[:, 0:4, :], et[:, 4:8, :])
            u1 = red.tile([P, 4, FN], BF16, tag="u", bufs=2, name="u1")
            nc.vector.tensor_add(u1, et[:, 8:12, :], et[:, 12:16, :])
            z_t = red.tile([P, 4, FN], BF16, tag="z", bufs=1, name="z_t")
            nc.vector.tensor_add(z_t, u0, u1)
            z2 = red.tile([P, 2, FN], BF16, tag="z2", bufs=1, name="z2")
            nc.vector.tensor_add(z2, z_t[:, 0:2], z_t[:, 2:4])
            dsum = red.tile([P, FN], BF16, tag="d", bufs=3, name="dsum")
            nc.vector.tensor_add(dsum, z2[:, 0], z2[:, 1])
            return dsum

        def emit_block(c, h, prev):
            """Scores+exp for unit (c,h), with the previous unit's 16 AV
            matmuls woven between the score j-pairs so the PE never waits on
            the ~1.16us/op exp stream (the scores PSUM tiles rotate only 3
            deep).  The denominator is an incremental DVE tree; the ones-
            matmul runs after the AV chain."""
            et = exp_pool.tile([P, SK, FN], BF16, name="et")
            if prev is not None:
                pc, ph, pet, pdsum = prev
                # 1-bank accumulator, double-buffered: block n+1's AV chain
                # does not wait for block n's normalize to drain
                ps_av = psum.tile([P, FN], F32, tag="av", bufs=2, name="ps_av")
            for j in range(8):
                emit_score_pair(c, h, j, et)
                if prev is not None:
                    for k in (2 * j, 2 * j + 1):
                        nc.tensor.matmul(
                            ps_av,
                            v_sb[:, k, ph * P:(ph + 1) * P],
                            pet[:, k, :],
                            start=(k == 0), stop=(k == SK - 1),
                        )
            # finish the previous unit FIRST: its reciprocal+normalize must
            # not sit behind this unit's tree in the DVE queue, or the single
            # AV accumulator stays pinned and stalls the next block
            if prev is not None:
                finish_av(ps_av, pc, ph, pdsum)
            dsum = emit_tree(et)
            return (c, h, et, dsum)

        def emit_av_tail(prev):
            pc, ph, pet, pdsum = prev
            ps_av = psum.tile([P, FN], F32, tag="av", bufs=2, name="ps_av")
            for k in range(SK):
                nc.tensor.matmul(
                    ps_av,
                    v_sb[:, k, ph * P:(ph + 1) * P],
                    pet[:, k, :],
                    start=(k == 0), stop=(k == SK - 1),
                )
            finish_av(ps_av, pc, ph, pdsum)

        def emit_ctile(m, act_copy=False):
            for c2p in range(2):
                ps = psum.tile([P, 2, FN], F32, tag="ps", name="ps_c")
                for half in range(2):
                    c2 = 2 * c2p + half
                    for kh in range(HG):
                        nc.tensor.matmul(
                            ps[:, half],
                            outt[:, kh, m * P:(m + 1) * P],
                            wo_s[:, kh, c2 * FN:(c2 + 1) * FN],
                            start=(kh == 0), stop=(kh == HG - 1),
                        )
                yt = ystage.tile([P, 2 * FN], BF16, name="yt")
                if act_copy and c2p == 1:
                    # tail only: the exp stream is finished, so ACT is idle
                    # and can stage half the output copies
                    nc.scalar.activation(
                        yt, ps.rearrange("p a q -> p (a q)"),
                        mybir.ActivationFunctionType.Copy)
                else:
                    nc.vector.tensor_copy(yt, ps.rearrange("p a q -> p (a q)"))
                nc.sync.dma_start(
                    y[m * P:(m + 1) * P, c2p * 2 * FN:(c2p + 1) * 2 * FN], yt)

        # --- phase A: projections ------------------------------------------
        with tc.tile_pool(name="xpool", bufs=1) as xpool:
            xts = xpool.tile([P, KT, S], BF16)
            xt_r = xt.rearrange("(a p) s -> p a s", p=P)

            with tc.tile_pool(name="wpool", bufs=1) as wpool:
                # wk, wq AND wv live in one pool for all of phase A: wv must
                # not reuse wk/wq's SBUF or its DMA would wait for the last
                # Q^T matmul (write-after-read) and stall the V projection
                wk_s = wpool.tile([P, KT, CW], BF16)
                wq_s = wpool.tile([P, KT, CW], BF16)
                wv_s = wpool.tile([P, KT, CW], BF16)
                wk_r = wk.rearrange("(a p) m -> p a m", p=P)
                wq_r = wq.rearrange("(a p) m -> p a m", p=P)
                wv_r = wv.rearrange("(a p) m -> p a m", p=P)
                for k in range(KT):
                    nc.sync.dma_start(wk_s[:, k], wk_r[:, k])
                    nc.sync.dma_start(xts[:, k], xt_r[:, k])
                for k in range(KT):
                    nc.sync.dma_start(wq_s[:, k], wq_r[:, k])
                for k in range(KT):
                    nc.sync.dma_start(wv_s[:, k], wv_r[:, k])
                nc.sync.dma_start(wo_s, wo.rearrange("(a p) n -> p a n", p=P))

                # A1: K^T heads 0,1 -- contraction-outer DMA chase across all
                # 8 PSUM banks; 8 matmuls fire per arriving (wk, xt) chunk
                part1 = [(m, c) for m in (0, 1) for c in range(SQ)]
                banks = []
                for _ in range(3):
                    t = psum.tile([P, 2, FN], F32, tag="ps", name="a1")
                    banks += [t[:, 0], t[:, 1]]
                for _ in range(2):
                    banks.append(psum.tile([P, FN], F32, tag="av", bufs=2,
                                           name="a1v"))
                for k in range(KT):
                    for gi, (m, c) in enumerate(part1):
                        nc.tensor.matmul(
                            banks[gi],
                            wk_s[:, k, m * P:(m + 1) * P],
                            xts[:, k, c * FN:(c + 1) * FN],
                            start=(k == 0), stop=(k == KT - 1),
                        )
                for gi, (m, c) in enumerate(part1):
                    nc.vector.tensor_scalar_add(
                        kt_sb[:, m, c * FN:(c + 1) * FN],
                        banks[gi], bk_s[:, m:m + 1])

                # A2: K^T heads 2,3 (dense, inputs resident)
                def proj_pair(w_s, b_s, dst, g0, g1):
                    ps = psum.tile([P, 2, FN], F32, tag="ps", name="ps_a")
                    for half, (m, c) in enumerate((g0, g1)):
                        for k in range(KT):
                            nc.tensor.matmul(
                                ps[:, half],
                                w_s[:, k, m * P:(m + 1) * P],
                                xts[:, k, c * FN:(c + 1) * FN],
                                start=(k == 0), stop=(k == KT - 1),
                            )
                    for half, (m, c) in enumerate((g0, g1)):
                        nc.vector.tensor_scalar_add(
                            dst[:, m, c * FN:(c + 1) * FN],
                            ps[:, half], b_s[:, m:m + 1])

                part2 = [(m, c) for m in (2, 3) for c in range(SQ)]
                for i in range(0, 8, 2):
                    proj_pair(wk_s, bk_s, kt_sb, part2[i], part2[i + 1])

                # A3: Q^T (all heads)
                partq = [(m, c) for m in range(HG) for c in range(SQ)]
                for i in range(0, 16, 2):
                    proj_pair(wq_s, bq_s, qt, partq[i], partq[i + 1])

                # A4: V projection, with unit (c=0,h=0)'s score pairs and
                # exp woven between the 8 V pair-chains -- the ACT exp stream
                # for the first attention unit is fully pre-filled by the
                # time phase B starts, at zero PE cost
                def v_pair(m0, m1):
                    ps = psum.tile([P, 2, FN], F32, tag="ps", name="ps_v")
                    for half, m in enumerate((m0, m1)):
                        for k in range(KT):
                            nc.tensor.matmul(
                                ps[:, half],
                                xts[:, k, m * P:(m + 1) * P],
                                wv_s[:, k, :],
                                start=(k == 0), stop=(k == KT - 1),
                            )
                    for half, m in enumerate((m0, m1)):
                        nc.vector.tensor_add(v_sb[:, m, :], ps[:, half], bv_row)

                for j in range(8):
                    v_pair(2 * j, 2 * j + 1)
                    emit_score_pair(0, 0, j, et0)

        # --- phases B+C, software-pipelined --------------------------------
        exp_pool = ctx.enter_context(tc.tile_pool(name="expt", bufs=3))
        red = ctx.enter_context(tc.tile_pool(name="red", bufs=1))
        rden_pool = ctx.enter_context(tc.tile_pool(name="rden", bufs=2))
        ystage = ctx.enter_context(tc.tile_pool(name="ystage", bufs=3))
        outt = ctx.enter_context(tc.tile_pool(name="outt", bufs=1)).tile(
            [P, HG, S], BF16)

        # emission order interleaves score units, AV units, and the previous
        # chunk's output-projection tiles so that (a) the ACT exp stream never
        # paces the PE through the 4-deep PSUM rotation and (b) every AV unit
        # starts after its exp tile is complete
        dsum0 = emit_tree(et0)
        prev = (0, 0, et0, dsum0)
        for c in range(SQ):
            for h in range(HG):
                if c == 0 and h == 0:
                    continue  # unit (0,0) was woven into the V projection
                prev = emit_block(c, h, prev)
                if h == 1 and c >= 1:
                    for m in range(4 * (c - 1), 4 * c):
                        emit_ctile(m)
        emit_av_tail(prev)
        for m in range(4 * (SQ - 1), 4 * SQ):
            emit_ctile(m, act_copy=True)


def _get_nc():
    if "nc" not in _CACHE:
        _CACHE["nc"] = _build_nc()
    return _CACHE["nc"]


def make_in_maps(X, Wq, bq, Wk, bk, Wv, bv, Wo, bo):
    bf16 = ml_dtypes.bfloat16
    scale = 1.0 / math.sqrt(D)
    X = np.asarray(X, dtype=np.float32)
    xt_b = [np.ascontiguousarray(X[b].T).astype(bf16) for b in range(B)]
    Wq = np.asarray(Wq, dtype=np.float32) * scale
    Wk = np.asarray(Wk, dtype=np.float32)
    Wv = np.asarray(Wv, dtype=np.float32)
    Wo = np.asarray(Wo, dtype=np.float32)
    bq = np.asarray(bq, dtype=np.float32) * scale
    bk = np.asarray(bk, dtype=np.float32)
    bv = np.asarray(bv, dtype=np.float32)
    in_maps = []
    for c in range(NC):
        b, g = divmod(c, GROUPS)
        sl = slice(g * CW, (g + 1) * CW)
        in_maps.append({
            "xt": xt_b[b],
            "wq": np.ascontiguousarray(Wq[:, sl]).astype(bf16),
            "wk": np.ascontiguousarray(Wk[:, sl]).astype(bf16),
            "wv": np.ascontiguousarray(Wv[:, sl]).astype(bf16),
            "wo": np.ascontiguousarray(Wo[sl, :]).astype(bf16),
            "bq": np.ascontiguousarray(bq[sl]),
            "bk": np.ascontiguousarray(bk[sl]),
            "bv": np.ascontiguousarray(bv[sl]),
        })
    return in_maps


def gather_output(results, bo):
    bo = np.asarray(bo, dtype=np.float32)
    out = np.empty((B, S, H), np.float32)
    for b in range(B):
        acc = results[b * GROUPS]["y"].astype(np.float32, copy=True)
        for g in range(1, GROUPS):
            acc += results[b * GROUPS + g]["y"]
        out[b] = acc + bo[None, :]
    return out


def kernel(X, Wq, bq, Wk, bk, Wv, bv, Wo, bo):
    from concourse.bass_utils import run_bass_kernel_spmd

    in_maps = make_in_maps(X, Wq, bq, Wk, bk, Wv, bv, Wo, bo)
    nc = _get_nc()
    res = run_bass_kernel_spmd(nc, in_maps, list(range(NC))).results
    return gather_output(res, bo)


# revision 24
# speedup vs baseline: 1.0165x; 1.0165x over previous
"""Multi-head self-attention on 8 Trainium2 NeuronCores.

Problem: X[2,2048,2048] -> MHA(16 heads, head_dim 128) -> [2,2048,2048].

Sharding: core c in 0..7 handles batch b = c // 4 and head-group g = c % 4
(4 heads = 512 hidden columns per core).  Each core computes
    Q^T,K^T,V  (its 512-column slice of the QKV projections)
    per-head attention (softmax without max-subtraction; inputs are bounded)
    partial output projection  out_heads @ Wo[512-slice, :]  -> [2048, 2048]
The 4 partial projections per batch are summed on the host (the tensor-
parallel "all-reduce" is done in numpy) and the output bias is added there.

The PE is pure column-rate-bound (~0.49 ns/col sustained; LDWEIGHTS and
per-instruction overhead fully hidden), so the schedule is built to keep
the PE column stream dense from t~0:

  - ~10 warm-up matmuls on memset data run while the first DMA chunks land,
    so the HAM clock-gate reaches 8/8 before real work starts.
  - Phase A starts with a "chase" pass: 8 K^T groups (heads 0,1 x all four
    S-chunks) accumulate in 8 PSUM banks with the contraction (k) loop
    OUTERMOST, so every arriving (wk,xt) chunk immediately feeds 8 matmuls.
    PE never has to wait for the full 10.5 MB input load.
  - Softmax denominators: an incremental DVE pair-add chase over the exp
    tiles followed by ONE ones-matmul per unit, emitted AFTER the AV
    accumulation (the tree never sits on the PE critical path).
  - Phase B runs as a lag-1 software pipeline: unit (c,h)'s 16 score
    matmuls are woven with the previous unit's 16 AV matmuls, so the PE
    never waits on the exp stream through the 3-deep PSUM rotation.
  - Phase C (output projection) tiles for chunk c-1 are emitted one unit
    later as additional PE filler.
"""

import math
import sys
from contextlib import ExitStack

import numpy as np

sys.path.insert(0, "/opt/trn_rl_repo")

import ml_dtypes  # noqa: E402

import concourse.bass as bass  # noqa: E402
import concourse.mybir as mybir  # noqa: E402
import concourse.tile as tile  # noqa: E402
from concourse import bacc  # noqa: E402

B, S, H = 2, 2048, 2048
HEADS, D = 16, 128
NC = 8
GROUPS = 4            # cores per batch (head-group parallel)
CW = H // GROUPS      # 512 hidden columns per core (4 heads)
HG = CW // D          # 4 heads per core
P = 128               # partitions
FN = 512              # matmul free-dim / psum bank (f32)
KT = H // P           # 16 contraction tiles for the projections
SQ = S // FN          # 4 query chunks of 512
SK = S // P           # 16 key tiles of 128
NWARM = 10            # HAM warm-up filler matmuls

BF16 = mybir.dt.bfloat16
F32 = mybir.dt.float32

_CACHE = {}


def _build_nc():
    nc = bacc.Bacc()
    xt = nc.dram_tensor("xt", [H, S], BF16, kind="ExternalInput")
    wq = nc.dram_tensor("wq", [H, CW], BF16, kind="ExternalInput")
    wk = nc.dram_tensor("wk", [H, CW], BF16, kind="ExternalInput")
    wv = nc.dram_tensor("wv", [H, CW], BF16, kind="ExternalInput")
    wo = nc.dram_tensor("wo", [CW, H], BF16, kind="ExternalInput")
    bq = nc.dram_tensor("bq", [CW], F32, kind="ExternalInput")
    bk = nc.dram_tensor("bk", [CW], F32, kind="ExternalInput")
    bv = nc.dram_tensor("bv", [CW], F32, kind="ExternalInput")
    y = nc.dram_tensor("y", [S, H], BF16, kind="ExternalOutput")

    with tile.TileContext(nc) as tc:
        _emit(nc, tc, xt[:], wq[:], wk[:], wv[:], wo[:], bq[:], bk[:], bv[:], y[:])
    nc.finalize()
    return nc


def _emit(nc, tc, xt, wq, wk, wv, wo, bq, bk, bv, y):
    with ExitStack() as ctx:
        consts = ctx.enter_context(tc.tile_pool(name="consts", bufs=1))
        qkv = ctx.enter_context(tc.tile_pool(name="qkv", bufs=1))
        wo_pool = ctx.enter_context(tc.tile_pool(name="wo", bufs=1))
        # shared PSUM pool: [P, 2, FN] f32 tiles (2 banks each); tag "ps"
        # rotates 3 buffers and tag "av" (the AV accumulator) holds the
        # remaining 2 banks
        psum = ctx.enter_context(tc.tile_pool(name="psum", bufs=3, space="PSUM"))

        # --- constants ------------------------------------------------------
        ones_s = consts.tile([P, P], BF16)
        nc.vector.memset(ones_s, 1.0)
        warm_rhs = consts.tile([P, FN], BF16)
        nc.vector.memset(warm_rhs, 0.001)
        bq_s = consts.tile([P, HG], F32)
        bk_s = consts.tile([P, HG], F32)
        with nc.allow_non_contiguous_dma(reason="tiny one-time bias load"):
            nc.gpsimd.dma_start(bq_s, bq.rearrange("(m p) -> p m", p=P))
            nc.gpsimd.dma_start(bk_s, bk.rearrange("(m p) -> p m", p=P))
        bv_row = consts.tile([P, CW], F32)
        nc.gpsimd.dma_start(bv_row, bv[None, :].to_broadcast([P, CW]))

        # --- HAM warm-up fillers (run while the first DMA chunks land) -----
        warm_ps = psum.tile([P, 2, FN], F32, tag="ps", name="warm_ps")
        for i in range(NWARM):
            nc.tensor.matmul(warm_ps[:, i % 2], ones_s, warm_rhs,
                             start=True, stop=True)

        qt = qkv.tile([P, HG, S], BF16)     # Q^T: [d-part, head, S]
        kt_sb = qkv.tile([P, HG, S], BF16)  # K^T
        v_sb = qkv.tile([P, SK, CW], BF16)  # V: [S-part(tile), S-tile, 4*d]
        et0 = qkv.tile([P, SK, FN], BF16)   # exp tile for unit (c=0, h=0)
        wo_s = wo_pool.tile([P, HG, H], BF16)

        # phase-B pools are opened lazily (SBUF is tight during phase A);
        # these names are captured by the closures below
        exp_pool = red = rden_pool = ystage = outt = None

        def finish_av(ps_av, pc, ph, pdsum):
            """ones-matmul + reciprocal + normalize for a completed AV chain."""
            nc.tensor.matmul(ps_av[:, 1], ones_s, pdsum, start=True, stop=True)
            rden = rden_pool.tile([P, FN], F32, name="rden")
            nc.vector.reciprocal_approx_fast(out=rden, in_=ps_av[:, 1])
            nc.vector.tensor_mul(
                outt[:, ph, pc * FN:(pc + 1) * FN], ps_av[:, 0], rden)

        def emit_score_pair(c, h, j, et):
            ps = psum.tile([P, 2, FN], F32, tag="ps", name="ps_s")
            for i in range(2):
                nc.tensor.matmul(
                    ps[:, i],
                    kt_sb[:, h, (2 * j + i) * P:(2 * j + i + 1) * P],
                    qt[:, h, c * FN:(c + 1) * FN],
                    start=True, stop=True,
                )
            nc.scalar.activation(
                et[:, 2 * j:2 * j + 2, :].rearrange("p a q -> p (a q)"),
                ps.rearrange("p a q -> p (a q)"),
                mybir.ActivationFunctionType.Exp)

        def emit_tree(et):
            """incremental DVE denominator tree over a full exp tile"""
            u0 = red.tile([P, 4, FN], BF16, tag="u", bufs=2, name="u0")
            nc.vector.tensor_add(u0, et[:, 0:4, :], et[:, 4:8, :])
            u1 = red.tile([P, 4, FN], BF16, tag="u", bufs=2, name="u1")
            nc.vector.tensor_add(u1, et[:, 8:12, :], et[:, 12:16, :])
            z_t = red.tile([P, 4, FN], BF16, tag="z", bufs=1, name="z_t")
            nc.vector.tensor_add(z_t, u0, u1)
            z2 = red.tile([P, 2, FN], BF16, tag="z2", bufs=1, name="z2")
            nc.vector.tensor_add(z2, z_t[:, 0:2], z_t[:, 2:4])
            dsum = red.tile([P, FN], BF16, tag="d", bufs=3, name="dsum")
            nc.vector.tensor_add(dsum, z2[:, 0], z2[:, 1])
            return dsum

        def emit_block(c, h, prev):
            """Scores+exp for unit (c,h), with the previous unit's 16 AV
            matmuls woven between the score j-pairs so the PE never waits on
            the ~1.16us/op exp stream (the scores PSUM tiles rotate only 3
            deep).  The denominator is an incremental DVE tree; the ones-
            matmul runs after the AV chain."""
            et = exp_pool.tile([P, SK, FN], BF16, name="et")
            if prev is not None:
                pc, ph, pet, pdsum = prev
                ps_av = psum.tile([P, 2, FN], F32, tag="av", bufs=1,
                                  name="ps_av")
            for j in range(8):
                emit_score_pair(c, h, j, et)
                if prev is not None:
                    for k in (2 * j, 2 * j + 1):
                        nc.tensor.matmul(
                            ps_av[:, 0],
                            v_sb[:, k, ph * P:(ph + 1) * P],
                            pet[:, k, :],
                            start=(k == 0), stop=(k == SK - 1),
                        )
            # finish the previous unit FIRST: its reciprocal+normalize must
            # not sit behind this unit's tree in the DVE queue, or the single
            # AV accumulator stays pinned and stalls the next block
            if prev is not None:
                finish_av(ps_av, pc, ph, pdsum)
            dsum = emit_tree(et)
            return (c, h, et, dsum)

        def emit_av_tail(prev):
            pc, ph, pet, pdsum = prev
            ps_av = psum.tile([P, 2, FN], F32, tag="av", bufs=1, name="ps_av")
            for k in range(SK):
                nc.tensor.matmul(
                    ps_av[:, 0],
                    v_sb[:, k, ph * P:(ph + 1) * P],
                    pet[:, k, :],
                    start=(k == 0), stop=(k == SK - 1),
                )
            finish_av(ps_av, pc, ph, pdsum)

        def emit_ctile(m, act_copy=False):
            for c2p in range(2):
                ps = psum.tile([P, 2, FN], F32, tag="ps", name="ps_c")
                for half in range(2):
                    c2 = 2 * c2p + half
                    for kh in range(HG):
                        nc.tensor.matmul(
                            ps[:, half],
                            outt[:, kh, m * P:(m + 1) * P],
                            wo_s[:, kh, c2 * FN:(c2 + 1) * FN],
                            start=(kh == 0), stop=(kh == HG - 1),
                        )
                yt = ystage.tile([P, 2 * FN], BF16, name="yt")
                if act_copy and c2p == 1:
                    # tail only: the exp stream is finished, so ACT is idle
                    # and can stage half the output copies
                    nc.scalar.activation(
                        yt, ps.rearrange("p a q -> p (a q)"),
                        mybir.ActivationFunctionType.Copy)
                else:
                    nc.vector.tensor_copy(yt, ps.rearrange("p a q -> p (a q)"))
                nc.sync.dma_start(
                    y[m * P:(m + 1) * P, c2p * 2 * FN:(c2p + 1) * 2 * FN], yt)

        # --- phase A: projections ------------------------------------------
        with tc.tile_pool(name="xpool", bufs=1) as xpool:
            xts = xpool.tile([P, KT, S], BF16)
            xt_r = xt.rearrange("(a p) s -> p a s", p=P)

            with tc.tile_pool(name="wpool", bufs=1) as wpool:
                # wk, wq AND wv live in one pool for all of phase A: wv must
                # not reuse wk/wq's SBUF or its DMA would wait for the last
                # Q^T matmul (write-after-read) and stall the V projection
                wk_s = wpool.tile([P, KT, CW], BF16)
                wq_s = wpool.tile([P, KT, CW], BF16)
                wv_s = wpool.tile([P, KT, CW], BF16)
                wk_r = wk.rearrange("(a p) m -> p a m", p=P)
                wq_r = wq.rearrange("(a p) m -> p a m", p=P)
                wv_r = wv.rearrange("(a p) m -> p a m", p=P)
                for k in range(KT):
                    nc.sync.dma_start(wk_s[:, k], wk_r[:, k])
                    nc.sync.dma_start(xts[:, k], xt_r[:, k])
                for k in range(KT):
                    nc.sync.dma_start(wq_s[:, k], wq_r[:, k])
                for k in range(KT):
                    nc.sync.dma_start(wv_s[:, k], wv_r[:, k])
                nc.sync.dma_start(wo_s, wo.rearrange("(a p) n -> p a n", p=P))

                # A1: K^T heads 0,1 -- contraction-outer DMA chase across all
                # 8 PSUM banks; 8 matmuls fire per arriving (wk, xt) chunk
                part1 = [(m, c) for m in (0, 1) for c in range(SQ)]
                a1 = [psum.tile([P, 2, FN], F32, tag="ps", name="a1")
                      for _ in range(3)]
                a1.append(psum.tile([P, 2, FN], F32, tag="av", bufs=1, name="a1v"))
                for k in range(KT):
                    for gi, (m, c) in enumerate(part1):
                        nc.tensor.matmul(
                            a1[gi // 2][:, gi % 2],
                            wk_s[:, k, m * P:(m + 1) * P],
                            xts[:, k, c * FN:(c + 1) * FN],
                            start=(k == 0), stop=(k == KT - 1),
                        )
                for gi, (m, c) in enumerate(part1):
                    nc.vector.tensor_scalar_add(
                        kt_sb[:, m, c * FN:(c + 1) * FN],
                        a1[gi // 2][:, gi % 2], bk_s[:, m:m + 1])

                # A2: K^T heads 2,3 (dense, inputs resident)
                def proj_pair(w_s, b_s, dst, g0, g1):
                    ps = psum.tile([P, 2, FN], F32, tag="ps", name="ps_a")
                    for half, (m, c) in enumerate((g0, g1)):
                        for k in range(KT):
                            nc.tensor.matmul(
                                ps[:, half],
                                w_s[:, k, m * P:(m + 1) * P],
                                xts[:, k, c * FN:(c + 1) * FN],
                                start=(k == 0), stop=(k == KT - 1),
                            )
                    for half, (m, c) in enumerate((g0, g1)):
                        nc.vector.tensor_scalar_add(
                            dst[:, m, c * FN:(c + 1) * FN],
                            ps[:, half], b_s[:, m:m + 1])

                part2 = [(m, c) for m in (2, 3) for c in range(SQ)]
                for i in range(0, 8, 2):
                    proj_pair(wk_s, bk_s, kt_sb, part2[i], part2[i + 1])

                # A3: Q^T (all heads)
                partq = [(m, c) for m in range(HG) for c in range(SQ)]
                for i in range(0, 16, 2):
                    proj_pair(wq_s, bq_s, qt, partq[i], partq[i + 1])

                # A4: V projection, with unit (c=0,h=0)'s score pairs and
                # exp woven between the 8 V pair-chains -- the ACT exp stream
                # for the first attention unit is fully pre-filled by the
                # time phase B starts, at zero PE cost
                def v_pair(m0, m1):
                    ps = psum.tile([P, 2, FN], F32, tag="ps", name="ps_v")
                    for half, m in enumerate((m0, m1)):
                        for k in range(KT):
                            nc.tensor.matmul(
                                ps[:, half],
                                xts[:, k, m * P:(m + 1) * P],
                                wv_s[:, k, :],
                                start=(k == 0), stop=(k == KT - 1),
                            )
                    for half, m in enumerate((m0, m1)):
                        nc.vector.tensor_add(v_sb[:, m, :], ps[:, half], bv_row)

                for j in range(8):
                    v_pair(2 * j, 2 * j + 1)
                    emit_score_pair(0, 0, j, et0)

        # --- phases B+C, software-pipelined --------------------------------
        exp_pool = ctx.enter_context(tc.tile_pool(name="expt", bufs=3))
        red = ctx.enter_context(tc.tile_pool(name="red", bufs=1))
        rden_pool = ctx.enter_context(tc.tile_pool(name="rden", bufs=2))
        ystage = ctx.enter_context(tc.tile_pool(name="ystage", bufs=3))
        outt = ctx.enter_context(tc.tile_pool(name="outt", bufs=1)).tile(
            [P, HG, S], BF16)

        # emission order interleaves score units, AV units, and the previous
        # chunk's output-projection tiles so that (a) the ACT exp stream never
        # paces the PE through the 4-deep PSUM rotation and (b) every AV unit
        # starts after its exp tile is complete
        dsum0 = emit_tree(et0)
        prev = (0, 0, et0, dsum0)
        for c in range(SQ):
            for h in range(HG):
                if c == 0 and h == 0:
                    continue  # unit (0,0) was woven into the V projection
                prev = emit_block(c, h, prev)
                if h == 1 and c >= 1:
                    for m in range(4 * (c - 1), 4 * c):
                        emit_ctile(m)
        emit_av_tail(prev)
        for m in range(4 * (SQ - 1), 4 * SQ):
            emit_ctile(m, act_copy=True)


def _get_nc():
    if "nc" not in _CACHE:
        _CACHE["nc"] = _build_nc()
    return _CACHE["nc"]


def make_in_maps(X, Wq, bq, Wk, bk, Wv, bv, Wo, bo):
    bf16 = ml_dtypes.bfloat16
    scale = 1.0 / math.sqrt(D)
    X = np.asarray(X, dtype=np.float32)
    xt_b = [np.ascontiguousarray(X[b].T).astype(bf16) for b in range(B)]
    Wq = np.asarray(Wq, dtype=np.float32) * scale
    Wk = np.asarray(Wk, dtype=np.float32)
    Wv = np.asarray(Wv, dtype=np.float32)
    Wo = np.asarray(Wo, dtype=np.float32)
    bq = np.asarray(bq, dtype=np.float32) * scale
    bk = np.asarray(bk, dtype=np.float32)
    bv = np.asarray(bv, dtype=np.float32)
    in_maps = []
    for c in range(NC):
        b, g = divmod(c, GROUPS)
        sl = slice(g * CW, (g + 1) * CW)
        in_maps.append({
            "xt": xt_b[b],
            "wq": np.ascontiguousarray(Wq[:, sl]).astype(bf16),
            "wk": np.ascontiguousarray(Wk[:, sl]).astype(bf16),
            "wv": np.ascontiguousarray(Wv[:, sl]).astype(bf16),
            "wo": np.ascontiguousarray(Wo[sl, :]).astype(bf16),
            "bq": np.ascontiguousarray(bq[sl]),
            "bk": np.ascontiguousarray(bk[sl]),
            "bv": np.ascontiguousarray(bv[sl]),
        })
    return in_maps


def gather_output(results, bo):
    bo = np.asarray(bo, dtype=np.float32)
    out = np.empty((B, S, H), np.float32)
    for b in range(B):
        acc = results[b * GROUPS]["y"].astype(np.float32, copy=True)
        for g in range(1, GROUPS):
            acc += results[b * GROUPS + g]["y"]
        out[b] = acc + bo[None, :]
    return out


def kernel(X, Wq, bq, Wk, bk, Wv, bv, Wo, bo):
    from concourse.bass_utils import run_bass_kernel_spmd

    in_maps = make_in_maps(X, Wq, bq, Wk, bk, Wv, bv, Wo, bo)
    nc = _get_nc()
    res = run_bass_kernel_spmd(nc, in_maps, list(range(NC))).results
    return gather_output(res, bo)
